# revision 25
# baseline (speedup 1.0000x reference)
"""FFM layer (linear + field-aware FM interaction) on 8 Trainium2 cores.

Sharding: row-parallel GEMM over the feature axis. Core c holds a
13056-feature stripe of inputs^T and of the combined weight matrix
G = [v.reshape(F, 312) | w], both bf16 (rel err ~0.3%, tolerance 2e-2).
Each core computes its partial inputs_c^T.T @ G_c -> [1024, 314] with
bf16 matmuls accumulated in fp32 PSUM over 102 k-tiles. The host sums
the 8 partials and applies the cheap FM epilogue (sum-square identity)
in fp64, returning [1024, 1] fp32.

Layout: one interleaved DRAM stream per core, [128, KT*1344] bf16.
Each k-tile slot holds [g(314) | pad(6) | xt(1024)] per partition, so
a chunk is one dma_start with large contiguous descriptors (n*2688 B
per partition) and 16B-aligned lhsT/rhs slices.

Timeline per trace analysis: PE-bound at ~160 ns per (LDWEIGHTS,
MATMUL) pair (314-cycle stream @2.4GHz + ~29 ns weight-swap bubble).
Warm-up LDWEIGHTS keep the PE HAM activity monitor busy during the
initial DMA wait so real matmuls start at 2.4 GHz. PSUM->SBUF copies
alternate vector/scalar engines; output goes out in 2 grouped DMAs.
"""

import numpy as np

B = 1024
F = 104013
FIELD = 39
K = 8
NV = FIELD * K          # 312 interaction columns
NL = NV                 # linear column index
NK = NV + 2             # + linear column + 1 zero pad col (even N: PSUM
                        # cachelines are 8B/two fp32 — odd N costs ~15%)
N_CORES = 8
KT = 102                # 128-row k-tiles per core
FPC = KT * 128          # 13056 padded features per core
GOFF = 320              # xt offset (elems) within a k-tile slot
SLOT = GOFF + B         # 1344 elems per k-tile slot (2688 B, 16B-aligned)
CH = 3                  # k-tiles per DMA chunk
BUFS = 8                # SBUF double-buffer depth for streamed chunks
WARM_MM = 50            # dummy matmuls before the stream (HAM pre-warm)
WARM_N = 64             # free dim of warm matmuls (small: fine-grained,
                        # drain never delays the first data-carrying MM)

_nc = None
last_exec_time_ns = None


def _build():
    from concourse import bass, mybir, tile, bacc

    nc = bacc.Bacc("TRN2", num_devices=N_CORES)
    f32 = mybir.dt.float32
    bf16 = mybir.dt.bfloat16

    xg = nc.dram_tensor("xg", [128, KT * SLOT], bf16, kind="ExternalInput")
    # Output stays partition-major ([128, 8*NK]: partition p, then batch
    # tile j, then column n) so the output DMAs have large contiguous
    # per-partition runs; the host untransposes. bf16 partials halve the
    # post-stream output-write bytes (rel err 0.27% vs 0.21%, budget 2%).
    out = nc.dram_tensor("out", [128, (B // 128) * NK], bf16, kind="ExternalOutput")

    with tile.TileContext(nc, pool_alloc_mode="queue") as tc:
        with (
            tc.tile_pool(name="xg", bufs=BUFS) as xg_pool,
            tc.tile_pool(name="acc", bufs=1, space=bass.MemorySpace.PSUM) as psum_pool,
            tc.tile_pool(name="o", bufs=1) as out_pool,
        ):
            n_b = B // 128
            accs = [
                psum_pool.tile([128, NK], f32, tag=f"acc{b}", name=f"acc{b}")
                for b in range(n_b)
            ]
            # Dummy matmuls on a zeroed tile keep the PE busy (HAM
            # activity monitor warm) while the first chunks stream in, so
            # the real matmuls run at 2.4 GHz from the start instead of
            # spending their first ~3.4us at 1.2. They write acc0 as a
            # self-contained start/stop group; the real k=0 matmul
            # (start=True) resets it.
            if WARM_MM:
                warm = out_pool.tile([128, 320], bf16, tag="warm", name="warm")
                nc.vector.memset(warm[:], 0.0)
                for _ in range(WARM_MM):
                    nc.tensor.matmul(
                        accs[0][:, :WARM_N],
                        warm[:, :128],
                        warm[:, :WARM_N],
                        start=True,
                        stop=True,
                    )
            # Graduated chunks: tiny first chunks so the PE starts as soon
            # as possible, steady CH-tile chunks afterwards, and a small
            # final chunk so the accs finish staggered (copy-out overlap).
            # All stream DMAs ride the sync HWDGE ring: splitting them
            # across the two rings was tried and cost 133->160 ns/matmul
            # (each MM then carries extra DMA-semaphore waits).
            chunks = [1, 1, 1, 1, 2, 2]
            while KT - sum(chunks) > 3:
                chunks.append(min(CH, KT - sum(chunks) - 3))
            chunks += [2, 1]
            kc = 0
            for ci, n in enumerate(chunks):
                last_chunk = ci == len(chunks) - 1
                t = xg_pool.tile([128, n * SLOT], bf16, tag="xg", name=f"xg{kc}")
                nc.sync.dma_start(t[:], xg[:, kc * SLOT : (kc + n) * SLOT])
                # b-major in the last chunk so each acc finishes (and its
                # copy-out can start) as early as possible.
                order = (
                    [(i, b) for b in range(n_b) for i in range(n)]
                    if last_chunk
                    else [(i, b) for i in range(n) for b in range(n_b)]
                )
                for i, b in order:
                    k = kc + i
                    nc.tensor.matmul(
                        accs[b][:],
                        t[:, i * SLOT + GOFF + b * 128 : i * SLOT + GOFF + (b + 1) * 128],
                        t[:, i * SLOT : i * SLOT + NK],
                        start=(k == 0),
                        stop=(k == KT - 1),
                    )
                kc += n
            # PSUM -> SBUF copies alternate vector/scalar (2x drain rate)
            # and downcast to bf16. Outputs leave in 3 DMAs: two on the
            # sync ring while the stream tail drains, then a single-acc
            # DMA on the act ring fired as soon as the last acc's copy
            # lands, so the exposed end-of-kernel DMA completion covers as
            # few bytes as possible.
            o = out_pool.tile([128, n_b * NK], bf16, tag="o", name="o")
            for b in range(n_b):
                if b % 2 == 0:
                    nc.vector.tensor_copy(o[:, b * NK : (b + 1) * NK], accs[b][:])
                else:
                    nc.scalar.copy(o[:, b * NK : (b + 1) * NK], accs[b][:])
                if b == 3:
                    nc.sync.dma_start(out[:, : 4 * NK], o[:, : 4 * NK])
                elif b == 6:
                    nc.sync.dma_start(out[:, 4 * NK : 7 * NK], o[:, 4 * NK : 7 * NK])
            nc.scalar.dma_start(out[:, 7 * NK :], o[:, 7 * NK :])
    nc.compile()
    return nc


def _get_nc():
    global _nc
    if _nc is None:
        _nc = _build()
    return _nc


def _pack_inputs(inputs, w, v):
    """Build per-core interleaved [128, KT*SLOT] bf16 streams."""
    import ml_dtypes

    bf16 = ml_dtypes.bfloat16
    FP = N_CORES * FPC
    XG = np.zeros((N_CORES, 128, KT, SLOT), dtype=bf16)
    # g part: [v | w] -> rows are features, cols are [312 v-cols, w, pad]
    Gv = XG[..., :NK].reshape(N_CORES, 128, KT, NK)
    G = np.zeros((FP, NK), dtype=bf16)
    G[:F, :NV] = v.reshape(F, NV).astype(bf16)
    G[:F, NL] = w[:, 0].astype(bf16)
    Gv[:] = G.reshape(N_CORES, KT, 128, NK).transpose(0, 2, 1, 3)
    # xt part: inputs^T
    XT = np.zeros((FP, B), dtype=bf16)
    XT[:F] = inputs.T.astype(bf16)
    XG[..., GOFF:] = XT.reshape(N_CORES, KT, 128, B).transpose(0, 2, 1, 3)
    return XG.reshape(N_CORES, 128, KT * SLOT)


def kernel(inputs, w0, w, v, _trace=False):
    global last_exec_time_ns
    from concourse.bass_utils import run_bass_kernel_spmd

    inputs = np.asarray(inputs, dtype=np.float32)
    w0 = np.asarray(w0, dtype=np.float32)
    w = np.asarray(w, dtype=np.float32)
    v = np.asarray(v, dtype=np.float32)

    XG = _pack_inputs(inputs, w, v)
    in_maps = [{"xg": XG[c]} for c in range(N_CORES)]
    nc = _get_nc()
    import os

    prev = os.environ.get("BASS_NEVER_TRACE")
    if not _trace:
        # Profiling needs an NTFF hook this container may not have; make
        # sure a stray BASS_TRACE env var can't pull us down that path.
        os.environ["BASS_NEVER_TRACE"] = "1"
    try:
        import time

        res = None
        for attempt in range(3):
            try:
                res = run_bass_kernel_spmd(
                    nc, in_maps, list(range(N_CORES)), trace=_trace
                )
                break
            except Exception:
                # Transient device wedges have been observed on shared
                # boxes; retry before giving up.
                if attempt == 2:
                    raise
                time.sleep(10)
    finally:
        if not _trace:
            if prev is None:
                os.environ.pop("BASS_NEVER_TRACE", None)
            else:
                os.environ["BASS_NEVER_TRACE"] = prev
    last_exec_time_ns = res.exec_time_ns

    total = np.zeros((B, NK), dtype=np.float64)
    for c in range(N_CORES):
        # device layout is [128, 8, NK] partition-major; batch row
        # r = j*128 + p lives at out[p, j*NK:(j+1)*NK]
        total += (
            res.results[c]["out"].reshape(128, B // 128, NK)
            .transpose(1, 0, 2)
            .reshape(B, NK)
        )

    field_f = total[:, :NV].reshape(B, FIELD, K)
    linear = total[:, NL] + np.float64(w0[0])
    s = field_f.sum(axis=1)                                     # [B, K]
    inter = 0.5 * ((s * s).sum(axis=-1) - (field_f * field_f).sum(axis=(1, 2)))
    return (linear + inter)[:, None].astype(np.float32)


# revision 26
# speedup vs baseline: 1.0034x; 1.0034x over previous
"""FFM layer (linear + field-aware FM interaction) on 8 Trainium2 cores.

Sharding: row-parallel GEMM over the feature axis. Core c holds a
13056-feature stripe of inputs^T and of the combined weight matrix
G = [v.reshape(F, 312) | w], both bf16 (rel err ~0.3%, tolerance 2e-2).
Each core computes its partial inputs_c^T.T @ G_c -> [1024, 314] with
bf16 matmuls accumulated in fp32 PSUM over 102 k-tiles. The host sums
the 8 partials and applies the cheap FM epilogue (sum-square identity)
in fp64, returning [1024, 1] fp32.

Layout: one interleaved DRAM stream per core, [128, KT*1344] bf16.
Each k-tile slot holds [g(314) | pad(6) | xt(1024)] per partition, so
a chunk is one dma_start with large contiguous descriptors (n*2688 B
per partition) and 16B-aligned lhsT/rhs slices.

Timeline per trace analysis: PE-bound at ~133 ns per (LDWEIGHTS,
MATMUL) pair = the 314-cycle rhs stream at 2.4 GHz; flat contiguous
2D weight slices keep the LDWEIGHTS fully hidden (3D-AP slices or
extra DMA-semaphore waits per MM cost ~160 ns/pair instead). Warm-up
matmuls keep the PE HAM activity monitor busy during the initial DMA
wait so real matmuls run at 2.4 GHz from the start. PSUM->SBUF copies
alternate vector/scalar engines; output leaves in 3 DMAs with only a
small single-acc DMA's completion latency exposed at the end.
"""

import numpy as np

B = 1024
F = 104013
FIELD = 39
K = 8
NV = FIELD * K          # 312 interaction columns
NL = NV                 # linear column index
NK = NV + 2             # + linear column + 1 zero pad col (even N: PSUM
                        # cachelines are 8B/two fp32 — odd N costs ~15%)
N_CORES = 8
KT = 102                # 128-row k-tiles per core
FPC = KT * 128          # 13056 padded features per core
GOFF = 320              # xt offset (elems) within a k-tile slot
SLOT = GOFF + B         # 1344 elems per k-tile slot (2688 B, 16B-aligned)
CH = 3                  # k-tiles per DMA chunk
BUFS = 8                # SBUF double-buffer depth for streamed chunks
WARM_MM = 50            # dummy matmuls before the stream (HAM pre-warm)
WARM_N = 64             # free dim of warm matmuls (small: fine-grained,
                        # drain never delays the first data-carrying MM)

_nc = None
last_exec_time_ns = None


def _build():
    from concourse import bass, mybir, tile, bacc

    nc = bacc.Bacc("TRN2", num_devices=N_CORES)
    f32 = mybir.dt.float32
    bf16 = mybir.dt.bfloat16

    xg = nc.dram_tensor("xg", [128, KT * SLOT], bf16, kind="ExternalInput")
    # Output stays partition-major ([128, 8*NK]: partition p, then batch
    # tile j, then column n) so the output DMAs have large contiguous
    # per-partition runs; the host untransposes. bf16 partials halve the
    # post-stream output-write bytes (rel err 0.27% vs 0.21%, budget 2%).
    out = nc.dram_tensor("out", [128, (B // 128) * NK], bf16, kind="ExternalOutput")

    with tile.TileContext(nc, pool_alloc_mode="queue") as tc:
        with (
            tc.tile_pool(name="xg", bufs=BUFS) as xg_pool,
            tc.tile_pool(name="acc", bufs=1, space=bass.MemorySpace.PSUM) as psum_pool,
            tc.tile_pool(name="o", bufs=1) as out_pool,
        ):
            n_b = B // 128
            accs = [
                psum_pool.tile([128, NK], f32, tag=f"acc{b}", name=f"acc{b}")
                for b in range(n_b)
            ]
            # Dummy matmuls on a zeroed tile keep the PE busy (HAM
            # activity monitor warm) while the first chunks stream in, so
            # the real matmuls run at 2.4 GHz from the start instead of
            # spending their first ~3.4us at 1.2. They write acc0 as a
            # self-contained start/stop group; the real k=0 matmul
            # (start=True) resets it.
            if WARM_MM:
                warm = out_pool.tile([128, 320], bf16, tag="warm", name="warm")
                nc.vector.memset(warm[:], 0.0)
                for _ in range(WARM_MM):
                    nc.tensor.matmul(
                        accs[0][:, :WARM_N],
                        warm[:, :128],
                        warm[:, :WARM_N],
                        start=True,
                        stop=True,
                    )
            # Graduated chunks: tiny first chunks so the PE starts as soon
            # as possible, steady CH-tile chunks afterwards, and a small
            # final chunk so the accs finish staggered (copy-out overlap).
            # All stream DMAs ride the sync HWDGE ring: splitting them
            # across the two rings was tried and cost 133->160 ns/matmul
            # (each MM then carries extra DMA-semaphore waits).
            chunks = [1, 1, 1, 1, 2, 2]
            while KT - sum(chunks) > 3:
                chunks.append(min(CH, KT - sum(chunks) - 3))
            chunks += [2, 1]
            kc = 0
            for ci, n in enumerate(chunks):
                last_chunk = ci == len(chunks) - 1
                t = xg_pool.tile([128, n * SLOT], bf16, tag="xg", name=f"xg{kc}")
                nc.sync.dma_start(t[:], xg[:, kc * SLOT : (kc + n) * SLOT])
                # b-major in the last chunk so each acc finishes (and its
                # copy-out can start) as early as possible.
                order = (
                    [(i, b) for b in range(n_b) for i in range(n)]
                    if last_chunk
                    else [(i, b) for i in range(n) for b in range(n_b)]
                )
                for i, b in order:
                    k = kc + i
                    nc.tensor.matmul(
                        accs[b][:],
                        t[:, i * SLOT + GOFF + b * 128 : i * SLOT + GOFF + (b + 1) * 128],
                        t[:, i * SLOT : i * SLOT + NK],
                        start=(k == 0),
                        stop=(k == KT - 1),
                    )
                kc += n
            # PSUM -> SBUF copies alternate vector/scalar (2x drain rate)
            # and downcast to bf16. Outputs leave in 3 DMAs: two on the
            # sync ring while the stream tail drains, then a single-acc
            # DMA on the act ring fired as soon as the last acc's copy
            # lands, so the exposed end-of-kernel DMA completion covers as
            # few bytes as possible.
            o = out_pool.tile([128, n_b * NK], bf16, tag="o", name="o")
            for b in range(n_b):
                if b % 2 == 0:
                    nc.vector.tensor_copy(o[:, b * NK : (b + 1) * NK], accs[b][:])
                else:
                    nc.scalar.copy(o[:, b * NK : (b + 1) * NK], accs[b][:])
                if b == 3:
                    nc.sync.dma_start(out[:, : 4 * NK], o[:, : 4 * NK])
                elif b == 6:
                    nc.sync.dma_start(out[:, 4 * NK : 7 * NK], o[:, 4 * NK : 7 * NK])
            nc.scalar.dma_start(out[:, 7 * NK :], o[:, 7 * NK :])
    nc.compile()
    return nc


def _get_nc():
    global _nc
    if _nc is None:
        _nc = _build()
    return _nc


def _pack_inputs(inputs, w, v):
    """Build per-core interleaved [128, KT*SLOT] bf16 streams."""
    import ml_dtypes

    bf16 = ml_dtypes.bfloat16
    FP = N_CORES * FPC
    XG = np.zeros((N_CORES, 128, KT, SLOT), dtype=bf16)
    # g part: [v | w] -> rows are features, cols are [312 v-cols, w, pad]
    Gv = XG[..., :NK].reshape(N_CORES, 128, KT, NK)
    G = np.zeros((FP, NK), dtype=bf16)
    G[:F, :NV] = v.reshape(F, NV).astype(bf16)
    G[:F, NL] = w[:, 0].astype(bf16)
    Gv[:] = G.reshape(N_CORES, KT, 128, NK).transpose(0, 2, 1, 3)
    # xt part: inputs^T
    XT = np.zeros((FP, B), dtype=bf16)
    XT[:F] = inputs.T.astype(bf16)
    XG[..., GOFF:] = XT.reshape(N_CORES, KT, 128, B).transpose(0, 2, 1, 3)
    return XG.reshape(N_CORES, 128, KT * SLOT)


def kernel(inputs, w0, w, v, _trace=False):
    global last_exec_time_ns
    from concourse.bass_utils import run_bass_kernel_spmd

    inputs = np.asarray(inputs, dtype=np.float32)
    w0 = np.asarray(w0, dtype=np.float32)
    w = np.asarray(w, dtype=np.float32)
    v = np.asarray(v, dtype=np.float32)

    XG = _pack_inputs(inputs, w, v)
    in_maps = [{"xg": XG[c]} for c in range(N_CORES)]
    nc = _get_nc()
    import os

    prev = os.environ.get("BASS_NEVER_TRACE")
    if not _trace:
        # Profiling needs an NTFF hook this container may not have; make
        # sure a stray BASS_TRACE env var can't pull us down that path.
        os.environ["BASS_NEVER_TRACE"] = "1"
    try:
        import time

        res = None
        for attempt in range(3):
            try:
                res = run_bass_kernel_spmd(
                    nc, in_maps, list(range(N_CORES)), trace=_trace
                )
                break
            except Exception:
                # Transient device wedges have been observed on shared
                # boxes; retry before giving up.
                if attempt == 2:
                    raise
                time.sleep(10)
    finally:
        if not _trace:
            if prev is None:
                os.environ.pop("BASS_NEVER_TRACE", None)
            else:
                os.environ["BASS_NEVER_TRACE"] = prev
    last_exec_time_ns = res.exec_time_ns

    total = np.zeros((B, NK), dtype=np.float64)
    for c in range(N_CORES):
        # device layout is [128, 8, NK] partition-major; batch row
        # r = j*128 + p lives at out[p, j*NK:(j+1)*NK]
        total += (
            res.results[c]["out"].reshape(128, B // 128, NK)
            .transpose(1, 0, 2)
            .reshape(B, NK)
        )

    field_f = total[:, :NV].reshape(B, FIELD, K)
    linear = total[:, NL] + np.float64(w0[0])
    s = field_f.sum(axis=1)                                     # [B, K]
    inter = 0.5 * ((s * s).sum(axis=-1) - (field_f * field_f).sum(axis=(1, 2)))
    return (linear + inter)[:, None].astype(np.float32)
